# revision 1
# baseline (speedup 1.0000x reference)
"""Trainium2 Bass kernel for nn_CascadedAttention (B=64, T=512, D=1024, V=28).

Math notes (why this is NOT a 512-step sequential scan on device):

  reference computes, per step t with carry y_prev (y_{-1} = 0):
    scores = softmax(tanh(...) @ Va, axis=-1)     # softmax over a SIZE-1 axis
                                                  # -> exactly 1.0 everywhere
    c      = einsum('btd,bt->bd', x, scores)      # -> x.sum(axis=1), step-invariant
    idx    = int32(y_prev)                        # y_prev in (0,1] -> idx in {0,1};
                                                  # idx==1 iff y_prev == 1.0 (fp32-saturated sigmoid)
    WoE    = emb_table[idx] @ Wo                  # -> w0 + (w1-w0)*idx elementwise
    y      = sigmoid(WoE + h_prev @ Uo + c @ Co)  # h_prev = x[:, t-1] (0 at t=0)

  So with G[b,t,v] = (x[b] @ Uo)[t,v], bias[b,v] = w0 + (c@Co)[b,v],
  delta = w1 - w0, and the binary state s_t = 1[G[t-1] + bias + delta*s_{t-1} >= theta]
  (theta = fp32 sigmoid saturation threshold; G[-1] := 0), the outputs are
      y_t = sigmoid(G[t-1] + bias + delta * s_{t-1}).
  s_t follows p0_t + (p1_t - p0_t)*s_{t-1} with p0_t = 1[G[t-1] >= theta-bias],
  p1_t = 1[G[t-1] >= theta-bias-delta], which maps exactly onto the DVE
  tensor_tensor_scan primitive (state = data0*state + data1): ONE instruction
  per batch-group. Wa, Ua, Va are mathematically dead (all-ones softmax).

Sharding: data-parallel over batch, 8 batches per core; x pre-transposed on
host to [BS, D, T] so every load is one contiguous [128, T] block.

Toolchain constraints that shaped the structure (nix walrus 2026-05):
  * ONE sync wait per instruction. Hence: warm-up consumers per engine for
    the constants, unique input tiles (no slot-recycling waits), a reserved
    DMA bookkeeping lane for the single output store (lane-first => its only
    wait is the sigmoid), and a patched Tile tail drain that splits its
    N-sem wait list into a chain of single-wait drains.
  * PE matmul psum writes only at partition bases {0, 32, 64}: two batches
    share a psum tile at bases 0/64 with stacked [Uo|pad|Co] weights.
"""

import numpy as np

import concourse.bass as bass
import concourse.mybir as mybir
import concourse.tile as _tile_mod
import concourse.tile_sem_assignment as _tsa
from concourse.tile import TileContext
from concourse.tile_scheduler import DMAInst
from concourse.vector_clock import ScopedClock
from concourse.bass_utils import run_bass_kernel_spmd

B, T, D, V = 64, 512, 1024, 28
N_CORES = 8
BS = B // N_CORES          # batches per core
KC = D // 128              # contraction chunks
NG = BS // 2               # psum pair-groups per core
F32 = mybir.dt.float32
# smallest fp32 x with 1/(1+exp(-x)) == 1.0 (24*ln2). Any value in [16, 19]
# yields indistinguishable outputs (see derivation above: a theta mismatch only
# flips idx where the NEXT sigmoid is saturated, shifting y by < 1e-6).
THETA = 16.635532333438687

CW = 64                    # packed weight chunk: 0:28 Uo, 32:60 Co, rest pad
WD = KC * CW               # column of [w0, delta, theta, theta-delta] scalars
NCONST = WD + 4

_NC_CACHE: dict = {}


# ---- Tile framework patches for the 1-wait-per-instruction walrus build ----

def _split_drain_and_barrier(self, tick_clock, wait_clock):
    """Tail drain: split its N-sem wait list into single-wait drains on SP."""
    nc = self.nc
    drain_inst = nc.sync.drain()
    wait_clock.add_sem_waits(
        drain_inst.ins, ScopedClock({None: tick_clock.global_clock})
    )
    si = drain_inst.ins.sync_info
    waits = list(si.on_wait) if si is not None and si.on_wait else []
    upds = list(si.on_update) if si is not None and si.on_update else []
    if len(waits) > 1:
        drain_inst.ins.sync_info = mybir.SyncInfo(on_wait=[waits[0]], on_update=[])
        for i, w in enumerate(waits[1:]):
            d2 = nc.sync.drain()
            last = i == len(waits) - 2
            d2.ins.sync_info = mybir.SyncInfo(
                on_wait=[w], on_update=upds if last else []
            )

    nc.all_engine_barrier()
    assert self.sems is not None
    popped = nc._tile_sem_poison_stack.pop()
    assert popped is self._sem_poison
    nc.clear_and_free_semaphores(list(self.sems.allocated().values()))
    nc.all_engine_barrier()


_tile_mod.TileContext._drain_and_barrier = _split_drain_and_barrier

# Reserve HWDGE bookkeeping lanes for the output stores (being lane-first,
# each store carries only its producer wait). All other HWDGE DMAs round-robin
# lanes 0-3.
_PIN_LANES: dict = {}
_orig_assign_tick = _tsa.TileClockTick._assign_tick


def _assign_tick_pin(self, inst):
    if isinstance(inst, DMAInst) and inst.engine != mybir.EngineType.Pool:
        if inst.name in _PIN_LANES:
            self.next_hw_dma_idx = _PIN_LANES[inst.name]
        elif self.next_hw_dma_idx >= 7:
            self.next_hw_dma_idx = 0
    return _orig_assign_tick(self, inst)


_tsa.TileClockTick._assign_tick = _assign_tick_pin


def _build_nc() -> bass.Bass:
    nc = bass.Bass()
    xt = nc.declare_dram_parameter("xt", [BS, D, T], F32, isOutput=False)
    consts = nc.declare_dram_parameter("consts", [128, NCONST], F32, isOutput=False)
    # output rows {0:28, 64:92} = batch {2g, 2g+1}, cols g*T+t; rest junk
    out = nc.declare_dram_parameter("out", [92, NG * T], F32, isOutput=True)

    with TileContext(nc) as tc:
        with (
            tc.tile_pool(name="consts_p", bufs=1) as cpool,
            tc.tile_pool(name="xin", bufs=1) as xpool,
            tc.tile_pool(name="mid", bufs=4) as mpool,
            tc.tile_pool(name="scan", bufs=2) as spool,
            tc.tile_pool(name="psum", bufs=NG, space="PSUM") as ppool,
        ):
            cb = cpool.tile([128, NCONST], F32)
            nc.sync.dma_start(out=cb[:], in_=consts[:])
            # DVE warm-up consumption so later DVE users carry no DMA wait
            junk = cpool.tile([1, 4], F32)
            nc.vector.tensor_copy(junk[:], cb[0:1, WD:WD + 4])

            # z for all 4 pair-groups side by side; zeroed so column g*T (the
            # t=0 slot) is 0 and junk rows stay finite
            z_all = cpool.tile([92, NG * T], F32)
            y_all = cpool.tile([92, NG * T], F32)
            nc.vector.memset(z_all[:], 0.0)

            ps_tiles = [
                ppool.tile([128, T], F32, tag="ps", name=f"ps{i}")
                for i in range(NG)
            ]
            # PE warm-up matmul consuming the consts DMA so no later matmul
            # needs more than one wait
            nc.tensor.matmul(
                ps_tiles[0][0:1, 0:1], cb[:, 0:1], cb[:, 0:1],
                start=True, stop=True,
            )

            # x loads: one [128, T] tile per (b, k), unique (no recycling
            # waits); 64 sequential 256 KiB direct2d transfers keep the DGE
            # ring dense at full HBM rate
            xk_tiles = {}
            for b in range(BS):
                for k in range(KC):
                    xk = xpool.tile(
                        [128, T], F32, tag=f"xk{b}_{k}", name=f"xk{b}_{k}"
                    )
                    nc.sync.dma_start(
                        out=xk[:], in_=xt[b, k * 128:(k + 1) * 128, :]
                    )
                    xk_tiles[b, k] = xk
                # one matmul per chunk: [Uo|pad|Co] stacked -> G rows at
                # base 64*(b%2), CC rows 32 above
                base = 64 * (b % 2)
                ps = ps_tiles[b // 2]
                for k in range(KC):
                    nc.tensor.matmul(
                        ps[base:base + CW, :],
                        cb[:, k * CW:(k + 1) * CW], xk_tiles[b, k][:],
                        start=(k == 0), stop=(k == KC - 1),
                    )

            for g in range(NG):
                ps = ps_tiles[g]
                zc = g * T     # this group's column block in z_all/y_all
                z0 = z_all[:, zc:zc + 1]  # always-zero column (memset)

                # bias[b] = w0 + sum_t CC.T: full-tile reduce, then shift the
                # CC rows (32:60, 96:124) down onto the G rows (0:28, 64:92)
                br = mpool.tile([124, 1], F32, tag="br")
                nc.vector.tensor_reduce(
                    out=br[:], in_=ps[0:124, :],
                    axis=mybir.AxisListType.X, op=mybir.AluOpType.add,
                )
                sb = mpool.tile([92, 1], F32, tag="sb")
                nc.vector.memset(sb[:], 0.0)
                nc.vector.tensor_copy(sb[0:28, :], br[32:60, :])
                nc.vector.tensor_copy(sb[64:92, :], br[96:124, :])
                nc.vector.tensor_scalar_add(sb[:], sb[:], cb[0:92, WD:WD + 1])
                # thresholds: tmb = theta - bias, tmbd = theta - bias - delta
                tmb = mpool.tile([92, 1], F32, tag="tmb")
                nc.vector.tensor_scalar(
                    out=tmb[:], in0=sb[:], scalar1=-1.0, scalar2=float(THETA),
                    op0=mybir.AluOpType.mult, op1=mybir.AluOpType.add,
                )
                tmbd = mpool.tile([92, 1], F32, tag="tmbd")
                nc.vector.tensor_scalar_sub(tmbd[:], tmb[:], cb[0:92, WD + 1:WD + 2])

                # p0/p1 indicators straight from psum (G rows; mid rows junk)
                p0 = spool.tile([92, T], F32, tag="p0")
                d01 = spool.tile([92, T], F32, tag="d01")
                bt = spool.tile([92, T], F32, tag="bt")
                nc.vector.tensor_scalar(
                    out=p0[:, 1:T], in0=ps[0:92, 0:T - 1], scalar1=tmb[:],
                    scalar2=None, op0=mybir.AluOpType.is_ge,
                )
                nc.vector.tensor_scalar(
                    out=p0[:, 0:1], in0=z0, scalar1=tmb[:],
                    scalar2=None, op0=mybir.AluOpType.is_ge,
                )
                nc.vector.tensor_scalar(
                    out=d01[:, 1:T], in0=ps[0:92, 0:T - 1], scalar1=tmbd[:],
                    scalar2=None, op0=mybir.AluOpType.is_ge,
                )
                nc.vector.tensor_copy(d01[:, 0:1], z0)  # any finite value
                nc.vector.tensor_sub(d01[:], d01[:], p0[:])
                # s_t = d01_t * s_{t-1} + p0_t   (exact on {0,1})
                nc.vector.tensor_tensor_scan(
                    out=bt[:], data0=d01[:], data1=p0[:], initial=0.0,
                    op0=mybir.AluOpType.mult, op1=mybir.AluOpType.add,
                )
                # z_t = G[t-1] + delta * s_{t-1}  (bias added by the sigmoid)
                nc.vector.scalar_tensor_tensor(
                    out=z_all[:, zc + 1:zc + T], in0=bt[:, 0:T - 1],
                    scalar=cb[0:92, WD + 1:WD + 2], in1=ps[0:92, 0:T - 1],
                    op0=mybir.AluOpType.mult, op1=mybir.AluOpType.add,
                )
                # y = sigmoid(z + bias)
                nc.scalar.activation(
                    out=y_all[:, zc:zc + T], in_=z_all[:, zc:zc + T],
                    func=mybir.ActivationFunctionType.Sigmoid,
                    bias=sb[:], scale=1.0,
                )
            st = nc.sync.dma_start(out=out[:], in_=y_all[:])
            _PIN_LANES[st.ins.name] = 7

    return nc


def _host_smalls(Wo, Uo, Co, emb_table):
    w0 = np.float32(emb_table[0].astype(np.float32) @ Wo[:, 0].astype(np.float32))
    w1 = np.float32(emb_table[1].astype(np.float32) @ Wo[:, 0].astype(np.float32))
    delta = np.float32(w1 - w0)
    theta = np.float32(THETA)
    uoco = np.zeros((D, CW), np.float32)
    uoco[:, 0:V] = Uo
    uoco[:, 32:32 + V] = Co
    consts = np.zeros((128, NCONST), np.float32)
    consts[:, 0:WD] = (
        uoco.reshape(KC, 128, CW).transpose(1, 0, 2).reshape(128, WD)
    )
    consts[:, WD:] = np.array(
        [w0, delta, theta, np.float32(theta - delta)], np.float32
    )
    return np.ascontiguousarray(consts)


def _in_maps(x, Wo, Uo, Co, emb_table):
    x = np.asarray(x, dtype=np.float32)
    consts = _host_smalls(
        np.asarray(Wo, np.float32), np.asarray(Uo, np.float32),
        np.asarray(Co, np.float32), np.asarray(emb_table, np.float32),
    )
    maps = []
    for c in range(N_CORES):
        xs = x[c * BS:(c + 1) * BS]                        # [BS, T, D]
        xtc = np.ascontiguousarray(xs.transpose(0, 2, 1))  # [BS, D, T]
        maps.append({"xt": xtc, "consts": consts})
    return maps


def _assemble(results):
    outs = []
    for c in range(len(results)):
        o = np.asarray(results[c]["out"]).reshape(92, NG, T)
        core = np.empty((BS, T, V), np.float32)
        core[0::2] = o[0:28].transpose(1, 2, 0)            # rows 0:28  = even b
        core[1::2] = o[64:92].transpose(1, 2, 0)           # rows 64:92 = odd b
        outs.append(core)
    return np.concatenate(outs, axis=0)                    # [B, T, V]


def _get_nc() -> bass.Bass:
    if "nc" not in _NC_CACHE:
        _NC_CACHE["nc"] = _build_nc()
    return _NC_CACHE["nc"]


def _run(inputs: dict, trace: bool = False):
    nc = _get_nc()
    maps = _in_maps(
        inputs["x"], inputs["Wo"], inputs["Uo"], inputs["Co"],
        inputs["emb_table"],
    )
    res = run_bass_kernel_spmd(nc, maps, list(range(N_CORES)), trace=trace)
    return res


def kernel(**inputs) -> np.ndarray:
    res = _run(inputs, trace=False)
    return _assemble(res.results)



# revision 13
# speedup vs baseline: 1.3686x; 1.3686x over previous
"""Trainium2 Bass kernel for nn_CascadedAttention (B=64, T=512, D=1024, V=28).

Math notes (why this is NOT a 512-step sequential scan on device):

  reference computes, per step t with carry y_prev (y_{-1} = 0):
    scores = softmax(tanh(...) @ Va, axis=-1)     # softmax over a SIZE-1 axis
                                                  # -> exactly 1.0 everywhere
    c      = einsum('btd,bt->bd', x, scores)      # -> x.sum(axis=1), step-invariant
    idx    = int32(y_prev)                        # y_prev in (0,1] -> idx in {0,1};
                                                  # idx==1 iff y_prev == 1.0 (fp32-saturated sigmoid)
    WoE    = emb_table[idx] @ Wo                  # -> w0 + (w1-w0)*idx elementwise
    y      = sigmoid(WoE + h_prev @ Uo + c @ Co)  # h_prev = x[:, t-1] (0 at t=0)

  So with G[b,t,v] = (x[b] @ Uo)[t,v], bias[b,v] = w0 + (c@Co)[b,v],
  delta = w1 - w0, and the binary state s_t = 1[G[t-1] + bias + delta*s_{t-1} >= theta]
  (theta = fp32 sigmoid saturation threshold; G[-1] := 0), the outputs are
      y_t = sigmoid(G[t-1] + bias + delta * s_{t-1}).
  With p0_t = 1[G[t-1] >= theta-bias], d1_t = 1[G[t-1] >= theta-bias-delta] and
  delta > 0 (so d1 >= p0), the state follows s_t = max(min(d1_t, s_{t-1}), p0_t),
  which maps onto the DVE tensor_tensor_scan primitive (op0=min, op1=max): ONE
  instruction per batch-group. Wa, Ua, Va are mathematically dead (all-ones
  softmax).

Precision: x and the [Uo|Co] weights are shipped fp16 (PSUM accumulation stays
fp32). Measured end-to-end max abs error vs the fp32 reference is 9.2e-3
(gate 2e-2); bf16 fails (6.8e-2) because bias = w0 + sum_t(x@Co) random-walks
the per-element rounding error over T*D. fp16 also makes the PE matmul 4x
faster (fp32 runs LOW/HIGH double-pass at 4 cycles/row; fp16 is 1) and halves
HBM traffic, the roofline for this kernel.

Sharding: data-parallel over batch, 8 batches per core; x pre-packed on host
to [BS, KC/2, 128, 2T] fp16 so every load is one contiguous [128, 2T] block
(2 KiB per partition line) and two matmul chunks share one DMA.

Toolchain constraints that shaped the structure (nix walrus 2026-05):
  * ONE sync wait per instruction. Hence: a PE warm-up matmul consuming the
    consts DMA, unique input tiles (no slot-recycling waits), per-group output
    stores on reserved lanes 4-7 (lane-first => each store's only wait is its
    sigmoid), the GpSimd d1 chain ordered so its is_ge carries only the PE
    wait (the preceding d1[:,0] copy reads tmbd, covering the DVE clock), and
    a patched Tile tail drain that splits its N-sem wait list into a chain of
    single-wait drains.
  * PE matmul psum writes only at partition bases {0, 32, 64}: two batches
    share a psum tile at bases 0/64 with stacked [Uo|pad|Co] weights.
  * x-load DMA triggers (~600 ns each on the issuing engine) round-robin over
    Sync/Vector/Scalar so no single engine's trigger stream throttles the
    23.5 us fp16 DMA window.
"""

import numpy as np

import concourse.bass as bass
import concourse.mybir as mybir
import concourse.tile as _tile_mod
import concourse.tile_sem_assignment as _tsa
from concourse.tile import TileContext
from concourse.tile_scheduler import DMAInst
from concourse.vector_clock import ScopedClock
from concourse.bass_utils import run_bass_kernel_spmd

B, T, D, V = 64, 512, 1024, 28
N_CORES = 8
BS = B // N_CORES          # batches per core
KC = D // 128              # contraction chunks
J = KC // 2                # DMA pair-chunks (2 contraction chunks per load)
NG = BS // 2               # psum pair-groups per core
F32 = mybir.dt.float32
F16 = mybir.dt.float16
# smallest fp32 x with 1/(1+exp(-x)) == 1.0 (24*ln2). Any value in [16, 19]
# yields indistinguishable outputs (a theta mismatch only flips the binary
# state where the NEXT sigmoid is saturated, shifting y by < 1e-6).
THETA = 16.635532333438687

CW = 64                    # packed weight chunk: 0:28 Uo, 32:60 Co, rest pad
WD = KC * CW

_NC_CACHE: dict = {}


# ---- Tile framework patches for the 1-wait-per-instruction walrus build ----

def _split_drain_and_barrier(self, tick_clock, wait_clock):
    """Tail drain: split its N-sem wait list into single-wait drains on SP."""
    nc = self.nc
    drain_inst = nc.sync.drain()
    wait_clock.add_sem_waits(
        drain_inst.ins, ScopedClock({None: tick_clock.global_clock})
    )
    si = drain_inst.ins.sync_info
    waits = list(si.on_wait) if si is not None and si.on_wait else []
    upds = list(si.on_update) if si is not None and si.on_update else []
    if len(waits) > 1:
        drain_inst.ins.sync_info = mybir.SyncInfo(on_wait=[waits[0]], on_update=[])
        for i, w in enumerate(waits[1:]):
            d2 = nc.sync.drain()
            last = i == len(waits) - 2
            d2.ins.sync_info = mybir.SyncInfo(
                on_wait=[w], on_update=upds if last else []
            )

    nc.all_engine_barrier()
    assert self.sems is not None
    popped = nc._tile_sem_poison_stack.pop()
    assert popped is self._sem_poison
    nc.clear_and_free_semaphores(list(self.sems.allocated().values()))
    nc.all_engine_barrier()


_tile_mod.TileContext._drain_and_barrier = _split_drain_and_barrier

# Reserve HWDGE bookkeeping lanes 4-7 for the per-group output stores (being
# lane-first, each store carries only its producer wait). All other HWDGE DMAs
# round-robin lanes 0-3 (each lane fans out to 4 of the 16 DMA engines, so 4
# lanes saturate the 358 GB/s per-core HBM read bandwidth).
_PIN_LANES: dict = {}
_orig_assign_tick = _tsa.TileClockTick._assign_tick


def _assign_tick_pin(self, inst):
    if isinstance(inst, DMAInst) and inst.engine != mybir.EngineType.Pool:
        if inst.name in _PIN_LANES:
            self.next_hw_dma_idx = _PIN_LANES[inst.name]
        elif self.next_hw_dma_idx >= 4:
            self.next_hw_dma_idx = 0
    return _orig_assign_tick(self, inst)


_tsa.TileClockTick._assign_tick = _assign_tick_pin


def _build_nc(w0: float, delta: float) -> bass.Bass:
    a_tmb = float(np.float32(THETA) - np.float32(w0))
    # negated d1 threshold, so d1 = sign(ps + ntmbd) on the Scalar engine
    a_ntmbd = float(np.float32(w0) + np.float32(delta) - np.float32(THETA))

    nc = bass.Bass()
    xt2 = nc.declare_dram_parameter("xt2", [BS, J, 128, 2 * T], F16, isOutput=False)
    consts = nc.declare_dram_parameter("consts", [128, WD], F16, isOutput=False)
    # per-group output: rows {0:28, 64:92} = batch {2g, 2g+1}, cols t
    out = nc.declare_dram_parameter("out", [NG, 92, T], F16, isOutput=True)

    trig = [nc.sync, nc.scalar]

    with TileContext(nc) as tc:
        with (
            tc.tile_pool(name="consts_p", bufs=1) as cpool,
            tc.tile_pool(name="xin", bufs=1) as xpool,
            tc.tile_pool(name="mid", bufs=1) as mpool,
            tc.tile_pool(name="scan", bufs=1) as spool,
            tc.tile_pool(name="psum", bufs=NG, space="PSUM") as ppool,
        ):
            cb = cpool.tile([128, WD], F16)
            nc.sync.dma_start(out=cb[:], in_=consts[:])

            # z staging for all groups; zeroed so column g*T (the t=0 slot)
            # is 0 and junk rows stay finite
            z_all = cpool.tile([92, NG * T], F32)
            nc.vector.memset(z_all[:], 0.0)

            ps_tiles = [
                ppool.tile([128, T], F32, tag="ps", name=f"ps{i}")
                for i in range(NG)
            ]
            # per-group threshold/bias scalars, zeroed up front so the junk
            # rows 28:64 read as initialized (their partitions are never used)
            sc_tiles = []
            for g in range(NG):
                sc = {
                    n: mpool.tile([92, 1], F32, tag=f"{n}{g}", name=f"{n}{g}")
                    for n in ("ntmbd", "tmb", "sb")
                }
                for t_ in sc.values():
                    nc.vector.memset(t_[:], 0.0)
                sc_tiles.append(sc)
            # PE warm-up matmul consuming the consts DMA so no later matmul
            # needs more than one wait
            nc.tensor.matmul(
                ps_tiles[0][0:1, 0:1], cb[:, 0:1], cb[:, 0:1],
                start=True, stop=True,
            )

            # x loads: one [128, 2T] fp16 tile per (b, j), unique (no
            # recycling waits); 32 sequential 256 KiB direct2d transfers on
            # lanes 0-3 keep the DGE rings dense at full HBM rate. Trigger
            # instructions round-robin Sync/Vector/Scalar.
            ti = 0
            for b in range(BS):
                base = 64 * (b % 2)
                ps = ps_tiles[b // 2]
                for j in range(J):
                    xj = xpool.tile(
                        [128, 2 * T], F16, tag=f"xj{b}_{j}", name=f"xj{b}_{j}"
                    )
                    trig[ti % 2].dma_start(out=xj[:], in_=xt2[b, j])
                    ti += 1
                    for h in range(2):
                        k = 2 * j + h
                        nc.tensor.matmul(
                            ps[base:base + CW, :],
                            cb[:, k * CW:(k + 1) * CW],
                            xj[:, h * T:(h + 1) * T],
                            start=(k == 0), stop=(k == KC - 1),
                        )

            for g in range(NG):
                ps = ps_tiles[g]
                zc = g * T     # this group's column block in z_all
                z0 = z_all[:, zc:zc + 1]  # always-zero column (memset)

                # bias row-sums: full-tile reduce (base-0 partition spans are
                # unrestricted; starting at 32 caps the span at 32), then
                # per-block remaps shift the CC sums (32:60, 96:124) onto the
                # G rows (0:28, 64:92). Rows 28:64 of the [92,1] scalars stay
                # uninitialized — they only feed junk partitions.
                br = mpool.tile([124, 1], F32, tag=f"br{g}")
                nc.vector.tensor_reduce(
                    out=br[:], in_=ps[0:124, :],
                    axis=mybir.AxisListType.X, op=mybir.AluOpType.add,
                )
                # thresholds: tmb = theta - bias (for p0's is_ge), ntmbd =
                # bias + delta - theta (negated, for d1's Sign activation);
                # ntmbd first so the Scalar engine's d1 chain unblocks earliest
                ntmbd = sc_tiles[g]["ntmbd"]
                tmb = sc_tiles[g]["tmb"]
                for dst, s1, a in ((ntmbd, 1.0, a_ntmbd), (tmb, -1.0, a_tmb)):
                    nc.vector.tensor_scalar(
                        out=dst[0:28], in0=br[32:60], scalar1=s1, scalar2=a,
                        op0=mybir.AluOpType.mult, op1=mybir.AluOpType.add,
                    )
                    nc.vector.tensor_scalar(
                        out=dst[64:92], in0=br[96:124], scalar1=s1, scalar2=a,
                        op0=mybir.AluOpType.mult, op1=mybir.AluOpType.add,
                    )

                # d1 = sign(ps - tmbd) on the Scalar engine, in parallel with
                # DVE's p0. sign's {-1,0,1} range is fine: in the min/max scan
                # any value <= 0 acts exactly like 0 (min(v,s)<=0 and
                # max(<=0, p0) = p0), and the 0-at-equality case only flips
                # the state where the next sigmoid is saturated. The d1[:,0]
                # seed copies ntmbd (any finite value works there) and hoists
                # the Scalar engine's DVE clock past ntmbd, so the big Sign
                # carries only the PE wait.
                d1 = spool.tile([92, T], F32, tag=f"d1{g}")
                nc.scalar.activation(
                    out=d1[:, 0:1], in_=ntmbd[:],
                    func=mybir.ActivationFunctionType.Copy,
                )
                nc.scalar.activation(
                    out=d1[:, 1:T], in_=ps[0:92, 0:T - 1],
                    func=mybir.ActivationFunctionType.Sign,
                    bias=ntmbd[:], scale=1.0,
                )

                p0 = spool.tile([92, T], F32, tag=f"p0{g}")
                nc.vector.tensor_scalar(
                    out=p0[:, 0:1], in0=z0, scalar1=tmb[:],
                    scalar2=None, op0=mybir.AluOpType.is_ge,
                )
                nc.vector.tensor_scalar(
                    out=p0[:, 1:T], in0=ps[0:92, 0:T - 1], scalar1=tmb[:],
                    scalar2=None, op0=mybir.AluOpType.is_ge,
                )
                # bias for the sigmoid (used only by ACT, emitted off the
                # scan's critical path)
                sb = sc_tiles[g]["sb"]
                nc.vector.tensor_scalar_add(sb[0:28], br[32:60], float(np.float32(w0)))
                nc.vector.tensor_scalar_add(sb[64:92], br[96:124], float(np.float32(w0)))

                # s_t = max(min(d1_t, s_{t-1}), p0_t)   (delta > 0; exact on {0,1})
                bt = spool.tile([92, T], F32, tag=f"bt{g}")
                nc.vector.tensor_tensor_scan(
                    out=bt[:], data0=d1[:], data1=p0[:], initial=0.0,
                    op0=mybir.AluOpType.min, op1=mybir.AluOpType.max,
                )
                # z_t = G[t-1] + delta * s_{t-1}  (bias added by the sigmoid)
                nc.vector.scalar_tensor_tensor(
                    out=z_all[:, zc + 1:zc + T], in0=bt[:, 0:T - 1],
                    scalar=float(np.float32(delta)), in1=ps[0:92, 0:T - 1],
                    op0=mybir.AluOpType.mult, op1=mybir.AluOpType.add,
                )
                # y = sigmoid(z + bias), downcast to fp16
                yg = spool.tile([92, T], F16, tag=f"y{g}")
                nc.scalar.activation(
                    out=yg[:], in_=z_all[:, zc:zc + T],
                    func=mybir.ActivationFunctionType.Sigmoid,
                    bias=sb[:], scale=1.0,
                )
                st = nc.sync.dma_start(out=out[g], in_=yg[:])
                _PIN_LANES[st.ins.name] = 4 + g

    return nc


def _host_smalls(Wo, Uo, Co, emb_table):
    w0 = np.float32(emb_table[0].astype(np.float32) @ Wo[:, 0].astype(np.float32))
    w1 = np.float32(emb_table[1].astype(np.float32) @ Wo[:, 0].astype(np.float32))
    delta = np.float32(w1 - w0)
    assert delta >= 0, "min/max scan formulation requires delta >= 0"
    uoco = np.zeros((D, CW), np.float32)
    uoco[:, 0:V] = Uo
    uoco[:, 32:32 + V] = Co
    consts = (
        uoco.reshape(KC, 128, CW).transpose(1, 0, 2).reshape(128, WD)
    ).astype(np.float16)
    return np.ascontiguousarray(consts), float(w0), float(delta)


def _in_maps(x, Wo, Uo, Co, emb_table):
    x = np.asarray(x, dtype=np.float32)
    consts, w0, delta = _host_smalls(
        np.asarray(Wo, np.float32), np.asarray(Uo, np.float32),
        np.asarray(Co, np.float32), np.asarray(emb_table, np.float32),
    )
    maps = []
    for c in range(N_CORES):
        xs = x[c * BS:(c + 1) * BS]                        # [BS, T, D]
        xtc = xs.transpose(0, 2, 1).astype(np.float16)     # [BS, D, T] fp16
        # pack 2 contraction chunks side by side: [BS, J, 128, 2T]
        xt2 = np.ascontiguousarray(
            xtc.reshape(BS, J, 2, 128, T)
            .transpose(0, 1, 3, 2, 4)
            .reshape(BS, J, 128, 2 * T)
        )
        maps.append({"xt2": xt2, "consts": consts})
    return maps, w0, delta


def _assemble(results):
    outs = []
    for c in range(len(results)):
        o = np.asarray(results[c]["out"]).astype(np.float32)  # [NG, 92, T]
        core = np.empty((BS, T, V), np.float32)
        core[0::2] = o[:, 0:28].transpose(0, 2, 1)             # rows 0:28  = even b
        core[1::2] = o[:, 64:92].transpose(0, 2, 1)            # rows 64:92 = odd b
        outs.append(core)
    return np.concatenate(outs, axis=0)                        # [B, T, V]


def _get_nc(w0: float, delta: float) -> bass.Bass:
    key = (round(w0, 9), round(delta, 9))
    if key not in _NC_CACHE:
        _NC_CACHE[key] = _build_nc(w0, delta)
    return _NC_CACHE[key]


def _run(inputs: dict, trace: bool = False):
    maps, w0, delta = _in_maps(
        inputs["x"], inputs["Wo"], inputs["Uo"], inputs["Co"],
        inputs["emb_table"],
    )
    nc = _get_nc(w0, delta)
    res = run_bass_kernel_spmd(nc, maps, list(range(N_CORES)), trace=trace)
    return res


def kernel(**inputs) -> np.ndarray:
    res = _run(inputs, trace=False)
    return _assemble(res.results)


# revision 14
# speedup vs baseline: 1.6029x; 1.1712x over previous
"""Trainium2 Bass kernel for nn_CascadedAttention (B=64, T=512, D=1024, V=28).

Math notes (why this is NOT a 512-step sequential scan on device):

  reference computes, per step t with carry y_prev (y_{-1} = 0):
    scores = softmax(tanh(...) @ Va, axis=-1)     # softmax over a SIZE-1 axis
                                                  # -> exactly 1.0 everywhere
    c      = einsum('btd,bt->bd', x, scores)      # -> x.sum(axis=1), step-invariant
    idx    = int32(y_prev)                        # y_prev in (0,1] -> idx in {0,1};
                                                  # idx==1 iff y_prev == 1.0 (fp32-saturated sigmoid)
    WoE    = emb_table[idx] @ Wo                  # -> w0 + (w1-w0)*idx elementwise
    y      = sigmoid(WoE + h_prev @ Uo + c @ Co)  # h_prev = x[:, t-1] (0 at t=0)

  So with G[b,t,v] = (x[b] @ Uo)[t,v], bias[b,v] = w0 + (c@Co)[b,v],
  delta = w1 - w0, and the binary state s_t = 1[G[t-1] + bias + delta*s_{t-1} >= theta]
  (theta = fp32 sigmoid saturation threshold; G[-1] := 0), the outputs are
      y_t = sigmoid(G[t-1] + bias + delta * s_{t-1}).
  With p0_t = 1[G[t-1] >= theta-bias], d1_t = 1[G[t-1] >= theta-bias-delta] and
  delta > 0 (so d1 >= p0), the state follows s_t = max(min(d1_t, s_{t-1}), p0_t),
  which maps onto the DVE tensor_tensor_scan primitive (op0=min, op1=max): ONE
  instruction per batch-group. Wa, Ua, Va are mathematically dead (all-ones
  softmax).

Precision: x and the [Uo|Co] weights are shipped fp16 (PSUM accumulation stays
fp32). Measured end-to-end max abs error vs the fp32 reference is 9.2e-3
(gate 2e-2); bf16 fails (6.8e-2) because bias = w0 + sum_t(x@Co) random-walks
the per-element rounding error over T*D. fp16 also makes the PE matmul 4x
faster (fp32 runs LOW/HIGH double-pass at 4 cycles/row; fp16 is 1) and halves
HBM traffic, the roofline for this kernel.

Sharding: data-parallel over batch, 8 batches per core; x pre-packed on host
to [BS, KC/2, 128, 2T] fp16 so every load is one contiguous [128, 2T] block
(2 KiB per partition line) and two matmul chunks share one DMA.

Toolchain constraints that shaped the structure (nix walrus 2026-05):
  * ONE sync wait per instruction. Hence: a PE warm-up matmul consuming the
    consts DMA, unique input tiles (no slot-recycling waits), per-group output
    stores on reserved lanes 4-7 (lane-first => each store's only wait is its
    sigmoid), the GpSimd d1 chain ordered so its is_ge carries only the PE
    wait (the preceding d1[:,0] copy reads tmbd, covering the DVE clock), and
    a patched Tile tail drain that splits its N-sem wait list into a chain of
    single-wait drains.
  * PE matmul psum writes only at partition bases {0, 32, 64}: two batches
    share a psum tile at bases 0/64 with stacked [Uo|pad|Co] weights.
  * x-load DMA triggers (~600 ns each on the issuing engine) round-robin over
    Sync/Vector/Scalar so no single engine's trigger stream throttles the
    23.5 us fp16 DMA window.
"""

import numpy as np

import concourse.bass as bass
import concourse.mybir as mybir
import concourse.tile as _tile_mod
import concourse.tile_sem_assignment as _tsa
from concourse.tile import TileContext
from concourse.tile_scheduler import DMAInst
from concourse.vector_clock import ScopedClock
from concourse.bass_utils import run_bass_kernel_spmd

B, T, D, V = 64, 512, 1024, 28
N_CORES = 8
BS = B // N_CORES          # batches per core
KC = D // 128              # contraction chunks
J = KC // 2                # DMA pair-chunks (2 contraction chunks per load)
NG = BS // 2               # psum pair-groups per core
F32 = mybir.dt.float32
F16 = mybir.dt.float16
# smallest fp32 x with 1/(1+exp(-x)) == 1.0 (24*ln2). Any value in [16, 19]
# yields indistinguishable outputs (a theta mismatch only flips the binary
# state where the NEXT sigmoid is saturated, shifting y by < 1e-6).
THETA = 16.635532333438687

CW = 64                    # packed weight chunk: 0:28 Uo, 32:60 Co, rest pad
WD = KC * CW

_NC_CACHE: dict = {}


# ---- Tile framework patches for the 1-wait-per-instruction walrus build ----

def _split_drain_and_barrier(self, tick_clock, wait_clock):
    """Tail drain: split its N-sem wait list into single-wait drains on SP."""
    nc = self.nc
    drain_inst = nc.sync.drain()
    wait_clock.add_sem_waits(
        drain_inst.ins, ScopedClock({None: tick_clock.global_clock})
    )
    si = drain_inst.ins.sync_info
    waits = list(si.on_wait) if si is not None and si.on_wait else []
    upds = list(si.on_update) if si is not None and si.on_update else []
    if len(waits) > 1:
        drain_inst.ins.sync_info = mybir.SyncInfo(on_wait=[waits[0]], on_update=[])
        for i, w in enumerate(waits[1:]):
            d2 = nc.sync.drain()
            last = i == len(waits) - 2
            d2.ins.sync_info = mybir.SyncInfo(
                on_wait=[w], on_update=upds if last else []
            )

    nc.all_engine_barrier()
    assert self.sems is not None
    popped = nc._tile_sem_poison_stack.pop()
    assert popped is self._sem_poison
    nc.clear_and_free_semaphores(list(self.sems.allocated().values()))
    nc.all_engine_barrier()


_tile_mod.TileContext._drain_and_barrier = _split_drain_and_barrier

# Reserve HWDGE bookkeeping lanes 6-7 for the two output stores (being
# lane-first, each store carries only its producer wait). All other HWDGE DMAs
# round-robin lanes 0-5: each lane allows only ONE in-flight transfer (the
# next trigger waits the previous completion), so 6 lanes x 4 DMA engines
# oversubscribe the 16 engines 1.5x and keep the 358 GB/s per-core HBM read
# bandwidth saturated across the trigger gaps.
_PIN_LANES: dict = {}
_orig_assign_tick = _tsa.TileClockTick._assign_tick


def _assign_tick_pin(self, inst):
    if isinstance(inst, DMAInst) and inst.engine != mybir.EngineType.Pool:
        if inst.name in _PIN_LANES:
            self.next_hw_dma_idx = _PIN_LANES[inst.name]
        elif self.next_hw_dma_idx >= 6:
            self.next_hw_dma_idx = 0
    return _orig_assign_tick(self, inst)


_tsa.TileClockTick._assign_tick = _assign_tick_pin


def _build_nc(w0: float, delta: float) -> bass.Bass:
    a_tmb = float(np.float32(THETA) - np.float32(w0))
    # negated d1 threshold, so d1 = sign(ps + ntmbd) on the Scalar engine
    a_ntmbd = float(np.float32(w0) + np.float32(delta) - np.float32(THETA))

    nc = bass.Bass()
    xt4 = nc.declare_dram_parameter("xt4", [BS, 2, 128, 4 * T], F16, isOutput=False)
    consts = nc.declare_dram_parameter("consts", [128, WD], F16, isOutput=False)
    # output rows {0:28, 64:92} = batch {2g, 2g+1}, cols g*T+t; rest junk
    out = nc.declare_dram_parameter("out", [92, NG * T], F16, isOutput=True)

    with TileContext(nc) as tc:
        with (
            tc.tile_pool(name="consts_p", bufs=1) as cpool,
            tc.tile_pool(name="xin", bufs=1) as xpool,
            tc.tile_pool(name="mid", bufs=1) as mpool,
            tc.tile_pool(name="scan", bufs=1) as spool,
            tc.tile_pool(name="psum", bufs=NG, space="PSUM") as ppool,
        ):
            cb = cpool.tile([128, WD], F16)
            nc.sync.dma_start(out=cb[:], in_=consts[:])

            # z staging for all groups; zeroed so column g*T (the t=0 slot)
            # is 0 and junk rows stay finite
            z_all = cpool.tile([92, NG * T], F32)
            nc.vector.memset(z_all[:], 0.0)
            y_a = cpool.tile([92, 3 * T], F16)
            y_b = cpool.tile([92, T], F16)

            ps_tiles = [
                ppool.tile([128, T], F32, tag="ps", name=f"ps{i}")
                for i in range(NG)
            ]
            # per-group threshold/bias scalars, zeroed up front so the junk
            # rows 28:64 read as initialized (their partitions are never used)
            sc_tiles = []
            for g in range(NG):
                sc = {
                    n: mpool.tile([92, 1], F32, tag=f"{n}{g}", name=f"{n}{g}")
                    for n in ("ntmbd", "tmb", "sb")
                }
                for t_ in sc.values():
                    nc.vector.memset(t_[:], 0.0)
                sc_tiles.append(sc)
            # PE warm-up matmul consuming the consts DMA so no later matmul
            # needs more than one wait
            nc.tensor.matmul(
                ps_tiles[0][0:1, 0:1], cb[:, 0:1], cb[:, 0:1],
                start=True, stop=True,
            )

            # x loads: one [128, 4T] fp16 tile per (b, q) half-batch,
            # unique (no recycling waits); 16 sequential 512 KiB direct2d
            # transfers on lanes 0-5. The first 6 triggers are lane-first
            # (no lane wait) and go on Scalar, which must be free for the
            # per-group Sign/Sigmoid work as soon as matmuls finish; the
            # rest (which block on their lane's previous transfer) stay on
            # Sync, whose remaining duties are timing-noncritical.
            ti = 0
            for b in range(BS):
                base = 64 * (b % 2)
                ps = ps_tiles[b // 2]
                for q in range(2):
                    xq = xpool.tile(
                        [128, 4 * T], F16, tag=f"xq{b}_{q}", name=f"xq{b}_{q}"
                    )
                    eng = nc.scalar if ti < 6 else nc.sync
                    eng.dma_start(out=xq[:], in_=xt4[b, q])
                    ti += 1
                    for h in range(4):
                        k = 4 * q + h
                        nc.tensor.matmul(
                            ps[base:base + CW, :],
                            cb[:, k * CW:(k + 1) * CW],
                            xq[:, h * T:(h + 1) * T],
                            start=(k == 0), stop=(k == KC - 1),
                        )

            for g in range(NG):
                ps = ps_tiles[g]
                zc = g * T     # this group's column block in z_all
                z0 = z_all[:, zc:zc + 1]  # always-zero column (memset)

                # bias row-sums: full-tile reduce (base-0 partition spans are
                # unrestricted; starting at 32 caps the span at 32), then
                # per-block remaps shift the CC sums (32:60, 96:124) onto the
                # G rows (0:28, 64:92). Rows 28:64 of the [92,1] scalars stay
                # uninitialized — they only feed junk partitions.
                br = mpool.tile([124, 1], F32, tag=f"br{g}")
                nc.vector.tensor_reduce(
                    out=br[:], in_=ps[0:124, :],
                    axis=mybir.AxisListType.X, op=mybir.AluOpType.add,
                )
                # thresholds: tmb = theta - bias (for p0's is_ge), ntmbd =
                # bias + delta - theta (negated, for d1's Sign activation);
                # ntmbd first so the Scalar engine's d1 chain unblocks earliest
                ntmbd = sc_tiles[g]["ntmbd"]
                tmb = sc_tiles[g]["tmb"]
                for dst, s1, a in ((ntmbd, 1.0, a_ntmbd), (tmb, -1.0, a_tmb)):
                    nc.vector.tensor_scalar(
                        out=dst[0:28], in0=br[32:60], scalar1=s1, scalar2=a,
                        op0=mybir.AluOpType.mult, op1=mybir.AluOpType.add,
                    )
                    nc.vector.tensor_scalar(
                        out=dst[64:92], in0=br[96:124], scalar1=s1, scalar2=a,
                        op0=mybir.AluOpType.mult, op1=mybir.AluOpType.add,
                    )

                # d1 = sign(ps - tmbd) on the Scalar engine, in parallel with
                # DVE's p0. sign's {-1,0,1} range is fine: in the min/max scan
                # any value <= 0 acts exactly like 0 (min(v,s)<=0 and
                # max(<=0, p0) = p0), and the 0-at-equality case only flips
                # the state where the next sigmoid is saturated. The d1[:,0]
                # seed copies ntmbd (any finite value works there) and hoists
                # the Scalar engine's DVE clock past ntmbd, so the big Sign
                # carries only the PE wait.
                d1 = spool.tile([92, T], F32, tag=f"d1{g}")
                nc.scalar.activation(
                    out=d1[:, 0:1], in_=ntmbd[:],
                    func=mybir.ActivationFunctionType.Copy,
                )
                nc.scalar.activation(
                    out=d1[:, 1:T], in_=ps[0:92, 0:T - 1],
                    func=mybir.ActivationFunctionType.Sign,
                    bias=ntmbd[:], scale=1.0,
                )

                p0 = spool.tile([92, T], F32, tag=f"p0{g}")
                nc.vector.tensor_scalar(
                    out=p0[:, 0:1], in0=z0, scalar1=tmb[:],
                    scalar2=None, op0=mybir.AluOpType.is_ge,
                )
                nc.vector.tensor_scalar(
                    out=p0[:, 1:T], in0=ps[0:92, 0:T - 1], scalar1=tmb[:],
                    scalar2=None, op0=mybir.AluOpType.is_ge,
                )
                # bias for the sigmoid (used only by ACT, emitted off the
                # scan's critical path)
                sb = sc_tiles[g]["sb"]
                nc.vector.tensor_scalar_add(sb[0:28], br[32:60], float(np.float32(w0)))
                nc.vector.tensor_scalar_add(sb[64:92], br[96:124], float(np.float32(w0)))

                # s_t = max(min(d1_t, s_{t-1}), p0_t)   (delta > 0; exact on {0,1})
                bt = spool.tile([92, T], F32, tag=f"bt{g}")
                nc.vector.tensor_tensor_scan(
                    out=bt[:], data0=d1[:], data1=p0[:], initial=0.0,
                    op0=mybir.AluOpType.min, op1=mybir.AluOpType.max,
                )
                # z_t = G[t-1] + delta * s_{t-1}  (bias added by the sigmoid)
                nc.vector.scalar_tensor_tensor(
                    out=z_all[:, zc + 1:zc + T], in0=bt[:, 0:T - 1],
                    scalar=float(np.float32(delta)), in1=ps[0:92, 0:T - 1],
                    op0=mybir.AluOpType.mult, op1=mybir.AluOpType.add,
                )
                # y = sigmoid(z + bias), downcast to fp16. Groups 0-2 share
                # one tile so a single store (1 producer wait, via the Scalar
                # sem ordering) covers them; group 3's store stands alone so
                # only ~94 KiB remains after the last sigmoid.
                ya, yc = (y_a, zc) if g < 3 else (y_b, 0)
                nc.scalar.activation(
                    out=ya[:, yc:yc + T], in_=z_all[:, zc:zc + T],
                    func=mybir.ActivationFunctionType.Sigmoid,
                    bias=sb[:], scale=1.0,
                )
                if g == 2:
                    st = nc.sync.dma_start(out=out[:, 0:3 * T], in_=y_a[:])
                    _PIN_LANES[st.ins.name] = 6
                elif g == 3:
                    st = nc.sync.dma_start(out=out[:, 3 * T:4 * T], in_=y_b[:])
                    _PIN_LANES[st.ins.name] = 7

    return nc


def _host_smalls(Wo, Uo, Co, emb_table):
    w0 = np.float32(emb_table[0].astype(np.float32) @ Wo[:, 0].astype(np.float32))
    w1 = np.float32(emb_table[1].astype(np.float32) @ Wo[:, 0].astype(np.float32))
    delta = np.float32(w1 - w0)
    assert delta >= 0, "min/max scan formulation requires delta >= 0"
    uoco = np.zeros((D, CW), np.float32)
    uoco[:, 0:V] = Uo
    uoco[:, 32:32 + V] = Co
    consts = (
        uoco.reshape(KC, 128, CW).transpose(1, 0, 2).reshape(128, WD)
    ).astype(np.float16)
    return np.ascontiguousarray(consts), float(w0), float(delta)


def _in_maps(x, Wo, Uo, Co, emb_table):
    x = np.asarray(x, dtype=np.float32)
    consts, w0, delta = _host_smalls(
        np.asarray(Wo, np.float32), np.asarray(Uo, np.float32),
        np.asarray(Co, np.float32), np.asarray(emb_table, np.float32),
    )
    maps = []
    for c in range(N_CORES):
        xs = x[c * BS:(c + 1) * BS]                        # [BS, T, D]
        xtc = xs.transpose(0, 2, 1).astype(np.float16)     # [BS, D, T] fp16
        # pack 4 contraction chunks side by side: [BS, 2, 128, 4T]
        xt4 = np.ascontiguousarray(
            xtc.reshape(BS, 2, 4, 128, T)
            .transpose(0, 1, 3, 2, 4)
            .reshape(BS, 2, 128, 4 * T)
        )
        maps.append({"xt4": xt4, "consts": consts})
    return maps, w0, delta


def _assemble(results):
    outs = []
    for c in range(len(results)):
        o = np.asarray(results[c]["out"]).astype(np.float32).reshape(92, NG, T)
        core = np.empty((BS, T, V), np.float32)
        core[0::2] = o[0:28].transpose(1, 2, 0)                # rows 0:28  = even b
        core[1::2] = o[64:92].transpose(1, 2, 0)               # rows 64:92 = odd b
        outs.append(core)
    return np.concatenate(outs, axis=0)                        # [B, T, V]


def _get_nc(w0: float, delta: float) -> bass.Bass:
    key = (round(w0, 9), round(delta, 9))
    if key not in _NC_CACHE:
        _NC_CACHE[key] = _build_nc(w0, delta)
    return _NC_CACHE[key]


def _run(inputs: dict, trace: bool = False):
    maps, w0, delta = _in_maps(
        inputs["x"], inputs["Wo"], inputs["Uo"], inputs["Co"],
        inputs["emb_table"],
    )
    nc = _get_nc(w0, delta)
    res = run_bass_kernel_spmd(nc, maps, list(range(N_CORES)), trace=trace)
    return res


def kernel(**inputs) -> np.ndarray:
    res = _run(inputs, trace=False)
    return _assemble(res.results)
